# revision 11
# baseline (speedup 1.0000x reference)
"""Trainium2 Bass kernel for a 2-layer GRU (S=512, B=64, H=IN=1024).

Strategy: pure data-parallel over batch — core c owns batches [8c, 8c+8),
so the recurrence needs no inter-core communication (per-step collectives
measured ~100x their documented floor here). Each core holds the full
weight set (bf16) in SBUF and streams it through the PE array every step.

Layouts (per core, b = 8 local batches, j = 8 hidden k-tiles of 128):
  hidden state  HF:[128, 64] fp32, col = 8*j + b, partition p -> h row 128j+p
  gate preacts  gh^T in PSUM as [128, 64] per gate, same (j, b) cols
  matmul        out^T[(g,j) M-tiles, b] = Wh^T stationary, h^T moving (N=8)
Matmuls run in bf16 with fp32 PSUM accumulation; the recurrent state stays
fp32, only matmul operands are rounded to bf16.
"""

import sys

sys.path.insert(0, "/opt/trn_rl_repo")

import ml_dtypes
import numpy as np

import concourse.bacc as bacc
import concourse.bass as bass
import concourse.mybir as mybir
import concourse.tile as tile
from concourse.bass_utils import run_bass_kernel_spmd

BF16 = mybir.dt.bfloat16
F32 = mybir.dt.float32
AF = mybir.ActivationFunctionType
ALU = mybir.AluOpType

B = 64
H = 1024
L = 2
NC = 8
KT = H // 128  # contraction k-tiles
MT = 3 * KT  # M-tiles (gate-major: m = g*8 + j) per layer
BL = B // NC  # local batch


def build_nc(S, has_bias):
    TOK = S * BL  # tokens per core for the gi GEMMs
    CH = min(512, TOK)  # tokens per GEMM chunk
    CHS = CH // BL  # steps per chunk
    NCH = TOK // CH  # chunks
    nc = bacc.Bacc("TRN2", target_bir_lowering=False, debug=False, num_devices=NC)

    # ---- I/O ----
    xT = nc.dram_tensor("xT", [H, TOK], BF16, kind="ExternalInput")
    wi_in = nc.dram_tensor("wi_sb", [128, L * MT * KT * 128], BF16, kind="ExternalInput")
    wh_in = nc.dram_tensor("wh_sb", [128, L * MT * KT * 128], BF16, kind="ExternalInput")
    bias_in = nc.dram_tensor("bias_sb", [128, L * MT], F32, kind="ExternalInput")
    bhn_in = nc.dram_tensor("bhn_sb", [128, L * BL * KT], F32, kind="ExternalInput")
    h0_in = nc.dram_tensor("h0_own", [L * 128, BL * KT], F32, kind="ExternalInput")
    out_seq = nc.dram_tensor("out_seq", [S * 128, BL * KT], F32, kind="ExternalOutput")
    out_hn = nc.dram_tensor("out_hn", [L * 128, BL * KT], F32, kind="ExternalOutput")

    with tile.TileContext(nc) as tc:
        with (
            tc.tile_pool(name="wpool", bufs=1) as wpool,
            tc.tile_pool(name="xchunk", bufs=2) as xchunk_pool,
            tc.tile_pool(name="wistream", bufs=3) as wistream_pool,
            tc.tile_pool(name="giout", bufs=4) as giout_pool,
            tc.tile_pool(name="gpsum", bufs=4, space="PSUM") as gpsum_pool,
            tc.tile_pool(name="rzpsum", bufs=2, space="PSUM") as rzpsum_pool,
            tc.tile_pool(name="npsum", bufs=2, space="PSUM") as npsum_pool,
            tc.tile_pool(name="gtile", bufs=4) as gtile_pool,
            tc.tile_pool(name="ew", bufs=3) as ew_pool,
            tc.tile_pool(name="hf", bufs=3) as hf_pool,
            tc.tile_pool(name="hb", bufs=3) as hb_pool,
            tc.tile_pool(name="dram", bufs=1, space="DRAM") as dram_pool,
        ):
            # ---- persistent SBUF (Wi streams from DRAM per chunk) ----
            WH = wpool.tile([128, L * MT * KT * 128], BF16, tag="wh")
            BIAS = wpool.tile([128, L * MT], F32, tag="bias")
            nc.sync.dma_start(out=WH[:], in_=wh_in[:])
            nc.sync.dma_start(out=BIAS[:], in_=bias_in[:])
            if has_bias:
                BHN = wpool.tile([128, L * BL * KT], F32, tag="bhn")
                nc.sync.dma_start(out=BHN[:], in_=bhn_in[:])

            # ---- DRAM intermediates ----
            gi_dram = [
                dram_pool.tile([S * 128, 192], BF16, tag=f"gi{l}", name=f"gi{l}")
                for l in range(L)
            ]
            y1T = dram_pool.tile([S * 128, BL * KT], BF16, tag="y1T", name="y1T")

            def wslice(wt, l, m, k):
                i = (l * MT + m) * KT + k
                return wt[:, i * 128 : (i + 1) * 128]

            def gi_gemm(l, rhs_dram, rhs_is_xT):
                """gi[l]^T = Wi[l] @ input^T (+bias): all 3H rows, local tokens."""
                for c in range(NCH):
                    xt = xchunk_pool.tile([128, KT * CH], BF16, tag="xc")
                    if rhs_is_xT:
                        nc.sync.dma_start(
                            out=xt[:].rearrange("p (k n) -> p k n", k=KT),
                            in_=rhs_dram[:, c * CH : (c + 1) * CH].rearrange(
                                "(k p) n -> p k n", p=128
                            ),
                        )
                    else:
                        # y1T: rows (t, p), cols (j, b); k-tile j = cols 8j..
                        rows = rhs_dram[c * CHS * 128 : (c + 1) * CHS * 128, :]
                        for k in range(KT):
                            nc.sync.dma_start(
                                out=xt[:, k * CH : (k + 1) * CH].rearrange(
                                    "p (t b) -> p t b", t=CHS
                                ),
                                in_=rows[:, k * BL : (k + 1) * BL].rearrange(
                                    "(t p) b -> p t b", p=128
                                ),
                            )
                    for m in range(MT):
                        g, j = divmod(m, KT)
                        wt = wistream_pool.tile([128, KT * 128], BF16, tag="wt")
                        i0 = (l * MT + m) * KT * 128
                        nc.sync.dma_start(
                            out=wt[:], in_=wi_in[:, i0 : i0 + KT * 128]
                        )
                        ps = gpsum_pool.tile([128, CH], F32, tag="gp")
                        for k in range(KT):
                            nc.tensor.matmul(
                                ps[:],
                                lhsT=wt[:, k * 128 : (k + 1) * 128],
                                rhs=xt[:, k * CH : (k + 1) * CH],
                                start=(k == 0),
                                stop=(k == KT - 1),
                            )
                        ot = giout_pool.tile([128, CH], BF16, tag="go")
                        nc.vector.tensor_scalar_add(
                            ot[:], ps[:], BIAS[:, l * MT + m : l * MT + m + 1]
                        )
                        # CH cols = CHS steps x 8 batch -> gi rows (t,p)
                        nc.sync.dma_start(
                            out=gi_dram[l][
                                c * CHS * 128 : (c + 1) * CHS * 128,
                                g * 64 + j * BL : g * 64 + (j + 1) * BL,
                            ].rearrange("(t p) b -> p t b", p=128),
                            in_=ot[:].rearrange("p (t b) -> p t b", t=CHS),
                        )

            def recurrence(l):
                HF = hf_pool.tile([128, BL * KT], F32, tag="hf")
                nc.sync.dma_start(out=HF[:], in_=h0_in[l * 128 : (l + 1) * 128, :])
                HB = hb_pool.tile([128, BL * KT], BF16, tag="hb")
                nc.gpsimd.tensor_copy(HB[:], HF[:])
                for t in range(S):
                    G = gtile_pool.tile([128, 192], BF16, tag="g")
                    nc.sync.dma_start(
                        out=G[:], in_=gi_dram[l][t * 128 : (t + 1) * 128, :]
                    )
                    P_rz = rzpsum_pool.tile([128, 128], F32, tag="prz")
                    P_n = npsum_pool.tile([128, 64], F32, tag="pn")
                    for m in range(2 * KT):  # r and z gates
                        g, j = divmod(m, KT)
                        for k in range(KT):
                            nc.tensor.matmul(
                                P_rz[:, g * 64 + j * BL : g * 64 + (j + 1) * BL],
                                lhsT=wslice(WH, l, m, k),
                                rhs=HB[:, k * BL : (k + 1) * BL],
                                start=(k == 0),
                                stop=(k == KT - 1),
                            )
                    for j in range(KT):  # n gate
                        for k in range(KT):
                            nc.tensor.matmul(
                                P_n[:, j * BL : (j + 1) * BL],
                                lhsT=wslice(WH, l, 2 * KT + j, k),
                                rhs=HB[:, k * BL : (k + 1) * BL],
                                start=(k == 0),
                                stop=(k == KT - 1),
                            )
                    # elementwise gates
                    A = ew_pool.tile([128, 128], F32, tag="a")
                    nc.vector.tensor_tensor(A[:], P_rz[:], G[:, 0:128], ALU.add)
                    RZ = ew_pool.tile([128, 128], F32, tag="rz")
                    nc.scalar.activation(RZ[:], A[:], AF.Sigmoid)
                    TN = ew_pool.tile([128, 64], F32, tag="tn")
                    if has_bias:
                        nc.vector.tensor_tensor(
                            TN[:], P_n[:], BHN[:, l * 64 : (l + 1) * 64], ALU.add
                        )
                        nc.vector.tensor_tensor(TN[:], TN[:], RZ[:, 0:64], ALU.mult)
                    else:
                        nc.vector.tensor_tensor(TN[:], P_n[:], RZ[:, 0:64], ALU.mult)
                    nc.vector.tensor_tensor(TN[:], TN[:], G[:, 128:192], ALU.add)
                    N = ew_pool.tile([128, 64], F32, tag="n")
                    nc.scalar.activation(N[:], TN[:], AF.Tanh)
                    # h' = n - z*n + z*h
                    W_ = ew_pool.tile([128, 64], F32, tag="w")
                    nc.vector.tensor_tensor(W_[:], RZ[:, 64:128], HF[:], ALU.mult)
                    M_ = ew_pool.tile([128, 64], F32, tag="m")
                    nc.vector.tensor_tensor(M_[:], RZ[:, 64:128], N[:], ALU.mult)
                    HFn = hf_pool.tile([128, BL * KT], F32, tag="hf")
                    nc.vector.tensor_tensor(HFn[:], N[:], M_[:], ALU.subtract)
                    nc.vector.tensor_tensor(HFn[:], HFn[:], W_[:], ALU.add)
                    HF = HFn
                    HBn = hb_pool.tile([128, BL * KT], BF16, tag="hb")
                    nc.gpsimd.tensor_copy(HBn[:], HF[:])
                    HB = HBn
                    if l == 0:
                        nc.sync.dma_start(
                            out=y1T[t * 128 : (t + 1) * 128, :], in_=HB[:]
                        )
                    else:
                        nc.sync.dma_start(
                            out=out_seq[t * 128 : (t + 1) * 128, :], in_=HF[:]
                        )
                nc.sync.dma_start(out=out_hn[l * 128 : (l + 1) * 128, :], in_=HF[:])

            import os as _os

            for _ in range(int(_os.environ.get("CC_REP", "1"))):
                gi_gemm(0, xT, True)
                recurrence(0)
                gi_gemm(1, y1T, False)
                recurrence(1)

    nc.compile()
    return nc


def _pack_weights(W):
    # W: [L, 3, H, H] (out_h, in_h) -> [128, L*MT*KT*128] bf16.
    # Block i = (l*MT + g*KT + j)*KT + k holds W[l, g, 128j:+128, 128k:+128]^T.
    Wb = W.reshape(L, 3, KT, 128, KT, 128)  # [l, g, j, m, k, kk]
    Wb = Wb.transpose(0, 1, 2, 4, 5, 3).reshape(L * MT * KT, 128, 128)
    return np.ascontiguousarray(
        Wb.transpose(1, 0, 2).reshape(128, L * MT * KT * 128)
    ).astype(ml_dtypes.bfloat16)


def _jb_layout(a):
    # a: [lead, BL, H] -> [lead, 128, KT, BL] with col = j*BL + b
    lead = a.shape[0]
    a = a.reshape(lead, BL, KT, 128)
    return np.ascontiguousarray(a.transpose(0, 3, 2, 1))


def kernel(x, h_0, W_i, W_h, b_i, b_h):
    x = np.asarray(x, dtype=np.float32)
    h_0 = np.asarray(h_0, dtype=np.float32)
    W_i = np.asarray(W_i, dtype=np.float32)
    W_h = np.asarray(W_h, dtype=np.float32)
    b_i = np.asarray(b_i, dtype=np.float32)
    b_h = np.asarray(b_h, dtype=np.float32)
    S = x.shape[0]
    has_bias = bool(np.any(b_i) or np.any(b_h))

    wi_sb = _pack_weights(W_i)
    wh_sb = _pack_weights(W_h)
    # bias per (l, m=(g,j)): fold b_h into b_i for r,z gates (exact)
    bias_sb = np.zeros((128, L * MT), np.float32)
    for l in range(L):
        for g in range(3):
            for j in range(KT):
                v = b_i[l, g, 128 * j : 128 * (j + 1)].copy()
                if g < 2:
                    v += b_h[l, g, 128 * j : 128 * (j + 1)]
                bias_sb[:, l * MT + g * KT + j] = v
    # bh_n as a [128, (j,b)] tile per layer
    bhn_sb = np.zeros((128, L * BL * KT), np.float32)
    for l in range(L):
        v = b_h[l, 2].reshape(KT, 128)  # [j, p]
        bhn_sb[:, l * 64 : (l + 1) * 64] = np.repeat(
            v.T[:, :, None], BL, axis=2
        ).reshape(128, KT * BL)

    in_maps = []
    for c in range(NC):
        bs = slice(BL * c, BL * (c + 1))
        x_c = x[:, bs, :].reshape(S * BL, H)
        xT = np.ascontiguousarray(x_c.T).astype(ml_dtypes.bfloat16)
        h0_own = _jb_layout(h_0[:, bs, :]).reshape(L * 128, KT * BL).astype(np.float32)
        in_maps.append(
            {
                "xT": xT,
                "wi_sb": wi_sb,
                "wh_sb": wh_sb,
                "bias_sb": bias_sb,
                "bhn_sb": bhn_sb,
                "h0_own": np.ascontiguousarray(h0_own),
                }
        )

    nc = build_nc(S, has_bias)
    res = run_bass_kernel_spmd(nc, in_maps, core_ids=list(range(NC)))

    def unpack(a, lead):
        # [lead*128, KT*BL] -> [lead, BL, H]
        a = a.reshape(lead, 128, KT, BL)
        return a.transpose(0, 3, 2, 1).reshape(lead, BL, H)

    seq = np.concatenate(
        [unpack(r["out_seq"], S) for r in res.results], axis=1
    ).astype(np.float32)
    hn = np.concatenate(
        [unpack(r["out_hn"], L) for r in res.results], axis=1
    ).astype(np.float32)
    return seq, hn
